# revision 3
# baseline (speedup 1.0000x reference)
"""Trainium2 Bass kernel for nn_LlamaAttention_48816598286577.

Llama attention with block-streaming sparse mask (sink=1 block, local
window=8 blocks, BLOCK=128), B=1 S=2048 H=4096, 32 q heads / 8 kv heads,
head_dim 128, non-interleaved RoPE.

Sharding: tensor-parallel over heads across 8 cores (4 q heads + 1 kv
head per core). Everything on-chip is computed in transposed layout
(head_dim on partitions) so no activation transposes are needed except
V. o_proj partials are summed with 8 pipelined ReduceScatter chunks;
the host stitches the row stripes back together.

All matmuls run in float32r (e8m11, full PE rate at N>=256). Host
pre-rounds all DMA'd matmul operands to the f32r grid; on-device
producers (DVE/ACT) write float32r outputs which the HW rounds.
"""

import functools
import numpy as np

import concourse.bass as bass
import concourse.mybir as mybir
import concourse.tile as tile
from concourse import bacc
from concourse.bass_utils import run_bass_kernel_spmd

# problem constants (hardcoded per contract)
B, S, H = 1, 2048, 4096
NQ, NKV, HD = 32, 8, 128
BLOCK = 128
NBLK = S // BLOCK          # 16
SINK_BLOCKS = 1
LOCAL_BLOCKS = 8
ROPE_BASE = 10000.0
N_CORES = 8
HQ = NQ // N_CORES         # 4 q heads per core
DQ = HQ * HD               # 512 q columns per core
SCALE = 1.0 / float(np.sqrt(HD))

KC = H // 128              # 32 contraction chunks for projections
NQUART = 4                 # S split into 4 quarters of 512 for projections
QW = S // NQUART           # 512
RS_CHUNKS = 8              # o_proj rows per ReduceScatter chunk = S / RS_CHUNKS
RS_ROWS = S // RS_CHUNKS   # 256
STRIPE = RS_ROWS // N_CORES  # 32 rows per core per chunk

F32 = mybir.dt.float32
F32R = mybir.dt.float32r

# Opt-in profiling plumbing (off by default; harness never touches these).
TRACE = False
TRACE_KW: dict = {}
LAST_RESULTS = None


def _strip_range(j: int) -> tuple[int, int]:
    """Query-column range covered by key-block j's strip."""
    if j == 0:
        return 0, S
    return j * BLOCK, min((j + LOCAL_BLOCKS) * BLOCK, S)


def _pair_js(i: int) -> list[int]:
    """Key blocks contributing to query pair i (blocks 2i, 2i+1)."""
    return sorted(set([0]) | set(range(max(0, 2 * i - 7), 2 * i + 2)))


def build_nc(rs: bool = True, repeat: int = 1):
    nc = bacc.Bacc(
        "TRN2", target_bir_lowering=False, debug=False, num_devices=N_CORES
    )
    hidT = nc.dram_tensor("hidT", [H, S], F32R, kind="ExternalInput").ap()
    wq = nc.dram_tensor("wq", [H, DQ], F32R, kind="ExternalInput").ap()
    wk = nc.dram_tensor("wk", [H, HD], F32R, kind="ExternalInput").ap()
    wv = nc.dram_tensor("wv", [H, HD], F32R, kind="ExternalInput").ap()
    wo = nc.dram_tensor("wo", [H, DQ], F32R, kind="ExternalInput").ap()
    cosF = nc.dram_tensor("cosF", [128, S], F32, kind="ExternalInput").ap()
    sinS = nc.dram_tensor("sinS", [128, S], F32, kind="ExternalInput").ap()
    tri = nc.dram_tensor("tri", [128, 128], F32R, kind="ExternalInput").ap()
    eye = nc.dram_tensor("eye", [128, 128], F32R, kind="ExternalInput").ap()
    onescol = nc.dram_tensor("onescol", [128, 1], F32R, kind="ExternalInput").ap()
    maskA = nc.dram_tensor("maskA", [128, 256], F32R, kind="ExternalInput").ap()
    out = nc.dram_tensor("out", [S, DQ], F32, kind="ExternalOutput").ap()

    with tile.TileContext(nc) as tc:
      for rep in range(repeat):
        with (
            tc.tile_pool(name=f"persist{rep}", bufs=1) as pp,
            tc.tile_pool(name=f"dram{rep}", bufs=1, space="DRAM") as dramp,
        ):
            qTr = [
                [
                    pp.tile([128, QW], F32R, tag=f"qTr{h}_{nq}", name=f"qTr{h}_{nq}")
                    for nq in range(NQUART)
                ]
                for h in range(HQ)
            ]
            kTr = [
                pp.tile([128, QW], F32R, tag=f"kTr{nq}", name=f"kTr{nq}")
                for nq in range(NQUART)
            ]
            vNat = [
                pp.tile([128, QW], F32R, tag=f"vNat{nq}", name=f"vNat{nq}")
                for nq in range(NQUART)
            ]
            tri_sb = pp.tile([128, 128], F32R, tag="tri", name="tri_sb")
            maskA_sb = pp.tile([128, 256], F32R, tag="maskA", name="maskA_sb")
            eye_sb = pp.tile([128, 128], F32R, tag="eye", name="eye_sb")
            ones_sb = pp.tile([128, 1], F32R, tag="ones", name="ones_sb")
            nc.sync.dma_start(tri_sb[:], tri[:])
            nc.sync.dma_start(maskA_sb[:], maskA[:])
            nc.sync.dma_start(eye_sb[:], eye[:])
            nc.sync.dma_start(ones_sb[:], onescol[:])

            # ---------------- Phase 1: QKV projections + RoPE + V transpose
            with (
                tc.tile_pool(name=f"proj_sb{rep}", bufs=1) as sp,
                tc.tile_pool(name=f"proj_stream{rep}", bufs=3) as stp,
                tc.tile_pool(name=f"proj_ps{rep}", bufs=1, space="PSUM") as pspp,
                tc.tile_pool(name=f"tr_ps{rep}", bufs=2, space="PSUM") as trpp,
            ):
                vT = sp.tile([128, S], F32R, tag="vT", name="vT")
                wq_t, wk_t, wv_t = [], [], []
                for c in range(KC):
                    crow = slice(c * 128, (c + 1) * 128)
                    tq = sp.tile([128, DQ], F32R, tag=f"wqc{c}", name=f"wqc{c}")
                    tk = sp.tile([128, HD], F32R, tag=f"wkc{c}", name=f"wkc{c}")
                    tv = sp.tile([128, HD], F32R, tag=f"wvc{c}", name=f"wvc{c}")
                    nc.sync.dma_start(tq[:], wq[crow, :])
                    nc.sync.dma_start(tk[:], wk[crow, :])
                    nc.sync.dma_start(tv[:], wv[crow, :])
                    wq_t.append(tq)
                    wk_t.append(tk)
                    wv_t.append(tv)

                for nq in range(NQUART):
                    ncols = slice(nq * QW, (nq + 1) * QW)
                    ps_q = [
                        pspp.tile([128, QW], F32, tag=f"psq{h}", name=f"psq{h}") for h in range(HQ)
                    ]
                    ps_k = pspp.tile([128, QW], F32, tag="psk", name="ps_k")
                    ps_v = pspp.tile([128, QW], F32, tag="psv", name="ps_v")
                    for c in range(KC):
                        crow = slice(c * 128, (c + 1) * 128)
                        hid_c = stp.tile([128, QW], F32R, tag="hid", name="hid_c")
                        nc.sync.dma_start(hid_c[:], hidT[crow, ncols])
                        st, sp_ = (c == 0), (c == KC - 1)
                        for h in range(HQ):
                            nc.tensor.matmul(
                                ps_q[h][:],
                                wq_t[c][:, h * HD : (h + 1) * HD],
                                hid_c[:],
                                start=st,
                                stop=sp_,
                            )
                        nc.tensor.matmul(
                            ps_k[:], wk_t[c][:], hid_c[:],
                            start=st, stop=sp_,
                        )
                        nc.tensor.matmul(
                            ps_v[:], wv_t[c][:], hid_c[:],
                            start=st, stop=sp_,
                        )

                    # V^T: plain evacuation (rounds to f32r)
                    nc.vector.tensor_copy(vT[:, ncols], ps_v[:])

                    cos_sb = stp.tile([128, QW], F32, tag="cos", name="cos_sb", bufs=2)
                    sin_sb = stp.tile([128, QW], F32, tag="sin", name="sin_sb", bufs=2)
                    nc.sync.dma_start(cos_sb[:], cosF[:, ncols])
                    nc.sync.dma_start(sin_sb[:], sinS[:, ncols])

                    # RoPE directly from PSUM for q heads and k
                    for ps_x, dstT in [(ps_k, kTr[nq])] + [
                        (ps_q[h], qTr[h][nq]) for h in range(HQ)
                    ]:
                        raw = sp.tile([128, QW], F32, tag="raw", name="raw", bufs=2)
                        nc.vector.tensor_copy(raw[:], ps_x[:])
                        swp = sp.tile([128, QW], F32, tag="swp", name="swp", bufs=2)
                        nc.sync.dma_start(swp[0:64, :], raw[64:128, :])
                        nc.sync.dma_start(swp[64:128, :], raw[0:64, :])
                        t1 = sp.tile([128, QW], F32, tag="t1", name="t1", bufs=2)
                        t2 = sp.tile([128, QW], F32, tag="t2", name="t2", bufs=2)
                        nc.vector.tensor_mul(t1[:], raw[:], cos_sb[:])
                        nc.vector.tensor_mul(t2[:], swp[:], sin_sb[:])
                        nc.vector.tensor_add(dstT[:], t1[:], t2[:])

                    # V natural blocks for this quarter (4 transposes of 128x128)
                    for jb in range(nq * QW // 128, (nq + 1) * QW // 128):
                        bcols = slice(jb * 128, (jb + 1) * 128)
                        lcols = slice((jb % 4) * 128, (jb % 4 + 1) * 128)
                        tr = trpp.tile([128, 128], F32R, tag="tr", name="tr")
                        nc.tensor.transpose(tr[:], vT[:, bcols], eye_sb[:])
                        nc.vector.tensor_copy(vNat[nq][:, lcols], tr[:])

            # ---------------- Phase 2+3 fused: per query pair (all heads),
            # attention -> o_proj chunk -> ReduceScatter, fully pipelined.
            with (
                tc.tile_pool(name=f"wo_sb{rep}", bufs=1) as wop,
                tc.tile_pool(name=f"e_sb{rep}", bufs=3) as ep,
                tc.tile_pool(name=f"att_sb{rep}", bufs=2) as asb,
                tc.tile_pool(name=f"ev_sb{rep}", bufs=3) as evp,
                tc.tile_pool(name=f"s_ps{rep}", bufs=2, space="PSUM") as spsp,
                tc.tile_pool(name=f"o_ps{rep}", bufs=2, space="PSUM") as opsp,
                tc.tile_pool(name=f"m_ps{rep}", bufs=2, space="PSUM") as mpsp,
                tc.tile_pool(name=f"op_ps{rep}", bufs=2, space="PSUM") as oppp,
            ):
                wo_t = []
                for c in range(KC):
                    tw = wop.tile([128, DQ], F32R, tag=f"woc{c}", name=f"woc{c}")
                    nc.sync.dma_start(tw[:], wo[c * 128 : (c + 1) * 128, :])
                    wo_t.append(tw)
                ag_ins = [
                    dramp.tile([DQ, 256], F32R, tag=f"agin{c}", name=f"agin{c}")
                    for c in range(RS_CHUNKS)
                ]
                ag_outs = [
                    dramp.tile(
                        [H, 256], F32R, tag=f"agout{c}", name=f"agout{c}",
                        addr_space="Shared",
                    )
                    for c in range(RS_CHUNKS)
                ]

                for i in range(NBLK // 2):
                    q0 = i * 256
                    js = _pair_js(i)
                    for h in range(HQ):
                        e_t = ep.tile(
                            [128, len(js) * 256], F32R, tag="e", name="e_t"
                        )
                        oT = opsp.tile([128, 256], F32, tag="oT", name="oT")
                        sm = mpsp.tile([1, 256], F32, tag="sm", name="sm")
                        for idx, j in enumerate(js):
                            left = (j == 0) or (j <= 2 * i <= j + 7)
                            right = (j == 0) or (j <= 2 * i + 1 <= j + 7)
                            qs = q0 if left else q0 + 128
                            qe = q0 + 256 if right else q0 + 128
                            w = qe - qs
                            ecols = slice(idx * 256, idx * 256 + w)
                            s_ps = spsp.tile([128, 256], F32, tag="sps", name="s_ps")
                            kq_, kc_ = j // 4, (j % 4) * 128
                            qq_ = qs // QW
                            nc.tensor.matmul(
                                s_ps[:, 0:w],
                                kTr[kq_][:, kc_ : kc_ + 128],
                                qTr[h][qq_][:, qs - qq_ * QW : qe - qq_ * QW],
                                start=True,
                                stop=True,
                            )
                            nc.scalar.activation(
                                e_t[:, ecols],
                                s_ps[:, 0:w],
                                mybir.ActivationFunctionType.Exp,
                                scale=SCALE,
                            )
                            if j == 2 * i:
                                nc.vector.tensor_mul(
                                    e_t[:, ecols], e_t[:, ecols], maskA_sb[:]
                                )
                            elif j == 2 * i + 1:
                                nc.vector.tensor_mul(
                                    e_t[:, ecols], e_t[:, ecols], tri_sb[:]
                                )
                            st, sp_ = (idx == 0), (idx == len(js) - 1)
                            nc.tensor.matmul(
                                oT[:, qs - q0 : qe - q0],
                                vNat[j // 4][:, (j % 4) * 128 : (j % 4 + 1) * 128],
                                e_t[:, ecols],
                                start=st,
                                stop=sp_,
                            )
                            nc.tensor.matmul(
                                sm[:, qs - q0 : qe - q0],
                                ones_sb[:],
                                e_t[:, ecols],
                                start=st,
                                stop=sp_,
                            )
                        r_sb = asb.tile([1, 256], F32, tag="r", name="r_sb")
                        nc.vector.reciprocal(r_sb[:], sm[:])
                        r_dram = dramp.tile(
                            [1, 256], F32, tag="r_dram", name="r_dram", bufs=3
                        )
                        nc.sync.dma_start(r_dram[:], r_sb[:])
                        rb = asb.tile([128, 256], F32, tag="rb", name="rb")
                        r_bcast = bass.AP(
                            tensor=r_dram.tensor,
                            offset=r_dram.offset,
                            ap=[[0, 128]] + list(r_dram.ap[1:]),
                        )
                        nc.sync.dma_start(out=rb[:], in_=r_bcast)
                        at_c = asb.tile(
                            [128, 256], F32R, tag=f"at{h}", name=f"at{h}"
                        )
                        nc.vector.tensor_mul(at_c[:], oT[:], rb[:])
                        nc.sync.dma_start(
                            ag_ins[i][h * 128 : (h + 1) * 128, :], at_c[:]
                        )

                    # AllGather this 256-query chunk of attnT across cores,
                    # then o_proj of my 512 output columns for these rows.
                    if rs:
                        nc.gpsimd.collective_compute(
                            "AllGather",
                            mybir.AluOpType.bypass,
                            replica_groups=[list(range(N_CORES))],
                            ins=[ag_ins[i].opt()],
                            outs=[ag_outs[i].opt()],
                        )
                        ag_src = ag_outs[i]
                    else:
                        nc.sync.dma_start(ag_outs[i][0:DQ, :], ag_ins[i][:])
                        ag_src = ag_outs[i]
                    ps01 = [
                        oppp.tile([128, 512], F32, tag=f"op{sb}", name=f"op{sb}", bufs=1)
                        for sb in range(2)
                    ]
                    for c in range(KC):
                        ag_sb = evp.tile(
                            [128, 256], F32R, tag="ag_sb", name="ag_sb", bufs=6
                        )
                        nc.sync.dma_start(
                            ag_sb[:], ag_src[c * 128 : (c + 1) * 128, :]
                        )
                        for sb in range(2):
                            nc.tensor.matmul(
                                ps01[sb][:],
                                ag_sb[:, sb * 128 : (sb + 1) * 128],
                                wo_t[c][:],
                                start=(c == 0),
                                stop=(c == KC - 1),
                            )
                    for sb in range(2):
                        ev = evp.tile([128, 512], F32, tag="ev", name="ev")
                        nc.vector.tensor_copy(ev[:], ps01[sb][:])
                        nc.sync.dma_start(
                            out[q0 + sb * 128 : q0 + (sb + 1) * 128, :], ev[:]
                        )
    nc.compile()
    return nc


def round_f32r(x: np.ndarray) -> np.ndarray:
    u = np.ascontiguousarray(x, dtype=np.float32).view(np.uint32)
    r = (u + 0x7FF + ((u >> 12) & 1)) & np.uint32(0xFFFFF000)
    return r.view(np.float32)


@functools.lru_cache(maxsize=1)
def _cached_nc():
    return build_nc(rs=True)


def _tables():
    pos = np.arange(S, dtype=np.float64)
    inv = 1.0 / (ROPE_BASE ** (np.arange(0, HD, 2, dtype=np.float64) / HD))  # [64]
    f = inv[:, None] * pos[None, :]                   # [64, S]
    cos = np.cos(f).astype(np.float32)
    sin = np.sin(f).astype(np.float32)
    cosF = np.concatenate([cos, cos], axis=0)         # [128, S]
    sinS = np.concatenate([-sin, sin], axis=0)        # [128, S]
    k_idx = np.arange(128)[:, None]
    q_idx = np.arange(128)[None, :]
    tri = (k_idx <= q_idx).astype(np.float32)         # [k, q] causal in-block
    eye = np.eye(128, dtype=np.float32)
    maskA = np.concatenate([tri, np.ones((128, 128), np.float32)], axis=1)
    return cosF, sinS, tri, eye, maskA


def kernel(hidden_states, wq, wk, wv, wo):
    nc = _cached_nc()
    hidT = round_f32r(np.ascontiguousarray(hidden_states.reshape(S, H).T))
    cosF, sinS, tri, eye, maskA = _tables()
    in_maps = []
    for c in range(N_CORES):
        in_maps.append(
            {
                "hidT": hidT,
                "wq": round_f32r(wq[:, c * DQ : (c + 1) * DQ]),
                "wk": round_f32r(wk[:, c * HD : (c + 1) * HD]),
                "wv": round_f32r(wv[:, c * HD : (c + 1) * HD]),
                "wo": round_f32r(wo[:, c * DQ : (c + 1) * DQ]),
                "cosF": cosF,
                "sinS": sinS,
                "tri": round_f32r(tri),
                "eye": round_f32r(eye),
                "onescol": np.ones((128, 1), dtype=np.float32),
                "maskA": round_f32r(maskA),
            }
        )
    kw = dict(trace=True, **TRACE_KW) if TRACE else {}
    res = run_bass_kernel_spmd(nc, in_maps, core_ids=list(range(N_CORES)), **kw)
    global LAST_RESULTS
    LAST_RESULTS = res
    full = np.concatenate(
        [res.results[r]["out"] for r in range(N_CORES)], axis=1
    )
    return full.reshape(B, S, H)



# revision 4
# speedup vs baseline: 1.1843x; 1.1843x over previous
"""Trainium2 Bass kernel for nn_LlamaAttention_48816598286577.

Llama attention with block-streaming sparse mask (sink=1 block, local
window=8 blocks, BLOCK=128), B=1 S=2048 H=4096, 32 q heads / 8 kv heads,
head_dim 128, non-interleaved RoPE.

Sharding: tensor-parallel over heads across 8 cores (4 q heads + 1 kv
head per core). Everything on-chip is computed in transposed layout
(head_dim on partitions) so no activation transposes are needed except
V. Per 256-query chunk, the per-core attention outputs are AllGathered
(bf16) and each core computes its 512 columns of o_proj.

All matmul operands are bf16 (measured on HW: f32r matmul runs at ~2
cycles/row and LDWEIGHTS at ~4 cycles/row; bf16 runs both at 1).
Accumulation stays f32 in PSUM; RoPE math is f32. The phase-2/3 loop is
software-pipelined: attention for chunk i+1 and o_proj for chunk i-1
both overlap the AllGather for chunk i.
"""

import functools
import numpy as np
import ml_dtypes

import concourse.bass as bass
import concourse.mybir as mybir
import concourse.tile as tile
from concourse import bacc
from concourse.bass_utils import run_bass_kernel_spmd

# problem constants (hardcoded per contract)
B, S, H = 1, 2048, 4096
NQ, NKV, HD = 32, 8, 128
BLOCK = 128
NBLK = S // BLOCK          # 16
SINK_BLOCKS = 1
LOCAL_BLOCKS = 8
ROPE_BASE = 10000.0
N_CORES = 8
HQ = NQ // N_CORES         # 4 q heads per core
DQ = HQ * HD               # 512 q columns per core
SCALE = 1.0 / float(np.sqrt(HD))

KC = H // 128              # 32 contraction chunks for projections
NQUART = 4                 # S split into 4 quarters of 512 for projections
QW = S // NQUART           # 512

F32 = mybir.dt.float32
BF16 = mybir.dt.bfloat16

# Opt-in profiling plumbing (off by default; harness never touches these).
TRACE = False
TRACE_KW: dict = {}
LAST_RESULTS = None


def _pair_js(i: int) -> list[int]:
    """Key blocks contributing to query pair i (blocks 2i, 2i+1)."""
    return sorted(set([0]) | set(range(max(0, 2 * i - 7), 2 * i + 2)))


def build_nc(rs: bool = True, repeat: int = 1):
    nc = bacc.Bacc(
        "TRN2", target_bir_lowering=False, debug=False, num_devices=N_CORES
    )
    hidT = nc.dram_tensor("hidT", [H, S], BF16, kind="ExternalInput").ap()
    wq = nc.dram_tensor("wq", [H, DQ], BF16, kind="ExternalInput").ap()
    wk = nc.dram_tensor("wk", [H, HD], BF16, kind="ExternalInput").ap()
    wv = nc.dram_tensor("wv", [H, HD], BF16, kind="ExternalInput").ap()
    wo = nc.dram_tensor("wo", [H, DQ], BF16, kind="ExternalInput").ap()
    cosF = nc.dram_tensor("cosF", [128, S], F32, kind="ExternalInput").ap()
    sinS = nc.dram_tensor("sinS", [128, S], F32, kind="ExternalInput").ap()
    tri = nc.dram_tensor("tri", [128, 128], BF16, kind="ExternalInput").ap()
    eye = nc.dram_tensor("eye", [128, 128], BF16, kind="ExternalInput").ap()
    onescol = nc.dram_tensor("onescol", [128, 1], BF16, kind="ExternalInput").ap()
    maskA = nc.dram_tensor("maskA", [128, 256], BF16, kind="ExternalInput").ap()
    out = nc.dram_tensor("out", [S, DQ], F32, kind="ExternalOutput").ap()

    with tile.TileContext(nc) as tc:
      for rep in range(repeat):
        with (
            tc.tile_pool(name=f"persist{rep}", bufs=1) as pp,
            tc.tile_pool(name=f"dram{rep}", bufs=1, space="DRAM") as dramp,
            tc.tile_pool(name=f"wo_sb{rep}", bufs=1) as wop,
        ):
            qTr = [
                [
                    pp.tile([128, QW], BF16, tag=f"qTr{h}_{nq}", name=f"qTr{h}_{nq}")
                    for nq in range(NQUART)
                ]
                for h in range(HQ)
            ]
            kTr = [
                pp.tile([128, QW], BF16, tag=f"kTr{nq}", name=f"kTr{nq}")
                for nq in range(NQUART)
            ]
            vNat = [
                pp.tile([128, QW], BF16, tag=f"vNat{nq}", name=f"vNat{nq}")
                for nq in range(NQUART)
            ]
            tri_sb = pp.tile([128, 128], BF16, tag="tri", name="tri_sb")
            maskA_sb = pp.tile([128, 256], BF16, tag="maskA", name="maskA_sb")
            eye_sb = pp.tile([128, 128], BF16, tag="eye", name="eye_sb")
            ones_sb = pp.tile([128, 1], BF16, tag="ones", name="ones_sb")
            nc.sync.dma_start(tri_sb[:], tri[:])
            nc.sync.dma_start(maskA_sb[:], maskA[:])
            nc.sync.dma_start(eye_sb[:], eye[:])
            nc.sync.dma_start(ones_sb[:], onescol[:])

            # wo prefetch (overlaps phase 1) + CC-stream warmup collective
            wo_t = []
            for c in range(KC):
                tw = wop.tile([128, DQ], BF16, tag=f"woc{c}", name=f"woc{c}")
                nc.sync.dma_start(tw[:], wo[c * 128 : (c + 1) * 128, :])
                wo_t.append(tw)
            if rs:
                warm_in = dramp.tile([16, 16], BF16, tag="warm_in", name="warm_in")
                warm_out = dramp.tile(
                    [128, 16], BF16, tag="warm_out", name="warm_out",
                    addr_space="Shared",
                )
                warm_src = pp.tile([16, 16], BF16, tag="warm_src", name="warm_src")
                nc.vector.memset(warm_src[:], 0.0)
                nc.sync.dma_start(warm_in[:], warm_src[:])
                nc.gpsimd.collective_compute(
                    "AllGather",
                    mybir.AluOpType.bypass,
                    replica_groups=[list(range(N_CORES))],
                    ins=[warm_in.opt()],
                    outs=[warm_out.opt()],
                )

            # ---------------- Phase 1: QKV projections + RoPE + V transpose
            with (
                tc.tile_pool(name=f"proj_sb{rep}", bufs=1) as sp,
                tc.tile_pool(name=f"proj_stream{rep}", bufs=3) as stp,
                tc.tile_pool(name=f"proj_ps{rep}", bufs=1, space="PSUM") as pspp,
                tc.tile_pool(name=f"tr_ps{rep}", bufs=2, space="PSUM") as trpp,
            ):
                vT = sp.tile([128, S], BF16, tag="vT", name="vT")
                wq_t, wk_t, wv_t = [], [], []
                for c in range(KC):
                    crow = slice(c * 128, (c + 1) * 128)
                    tq = sp.tile([128, DQ], BF16, tag=f"wqc{c}", name=f"wqc{c}")
                    tk = sp.tile([128, HD], BF16, tag=f"wkc{c}", name=f"wkc{c}")
                    tv = sp.tile([128, HD], BF16, tag=f"wvc{c}", name=f"wvc{c}")
                    nc.sync.dma_start(tq[:], wq[crow, :])
                    nc.sync.dma_start(tk[:], wk[crow, :])
                    nc.sync.dma_start(tv[:], wv[crow, :])
                    wq_t.append(tq)
                    wk_t.append(tk)
                    wv_t.append(tv)

                for nq in range(NQUART):
                    ncols = slice(nq * QW, (nq + 1) * QW)
                    ps_q = [
                        pspp.tile([128, QW], F32, tag=f"psq{h}", name=f"psq{h}") for h in range(HQ)
                    ]
                    ps_k = pspp.tile([128, QW], F32, tag="psk", name="ps_k")
                    ps_v = pspp.tile([128, QW], F32, tag="psv", name="ps_v")
                    for c in range(KC):
                        crow = slice(c * 128, (c + 1) * 128)
                        hid_c = stp.tile([128, QW], BF16, tag="hid", name="hid_c")
                        nc.sync.dma_start(hid_c[:], hidT[crow, ncols])
                        st, sp_ = (c == 0), (c == KC - 1)
                        for h in range(HQ):
                            nc.tensor.matmul(
                                ps_q[h][:],
                                wq_t[c][:, h * HD : (h + 1) * HD],
                                hid_c[:],
                                start=st,
                                stop=sp_,
                            )
                        nc.tensor.matmul(
                            ps_k[:], wk_t[c][:], hid_c[:],
                            start=st, stop=sp_,
                        )
                        nc.tensor.matmul(
                            ps_v[:], wv_t[c][:], hid_c[:],
                            start=st, stop=sp_,
                        )

                    cos_sb = stp.tile([128, QW], F32, tag="cos", name="cos_sb", bufs=2)
                    sin_sb = stp.tile([128, QW], F32, tag="sin", name="sin_sb", bufs=2)
                    nc.sync.dma_start(cos_sb[:], cosF[:, ncols])
                    nc.sync.dma_start(sin_sb[:], sinS[:, ncols])

                    # Evacuate PSUM on ACT (frees banks fast; q0 first so the
                    # next quarter's first matmul unblocks earliest), then
                    # RoPE on DVE + swap via DMA.
                    rope_list = [(ps_q[h], qTr[h][nq], f"q{h}") for h in range(HQ)]
                    rope_list.append((ps_k, kTr[nq], "k"))
                    raws = []
                    for ps_x, dstT, tag in rope_list:
                        raw = sp.tile([128, QW], F32, tag=f"raw{tag}", name=f"raw{tag}")
                        nc.scalar.copy(raw[:], ps_x[:])
                        raws.append(raw)
                    nc.scalar.copy(vT[:, ncols], ps_v[:])
                    swps = []
                    for raw, (_, _, tag) in zip(raws, rope_list):
                        swp = sp.tile([128, QW], F32, tag=f"swp{tag}", name=f"swp{tag}")
                        nc.sync.dma_start(swp[0:64, :], raw[64:128, :])
                        nc.sync.dma_start(swp[64:128, :], raw[0:64, :])
                        swps.append(swp)
                    t1s = []
                    for raw, (_, _, tag) in zip(raws, rope_list):
                        t1 = sp.tile([128, QW], F32, tag=f"t1{tag}", name=f"t1{tag}")
                        nc.vector.tensor_mul(t1[:], raw[:], cos_sb[:])
                        t1s.append(t1)
                    for t1, swp, (_, dstT, _) in zip(t1s, swps, rope_list):
                        nc.vector.tensor_mul(swp[:], swp[:], sin_sb[:])
                        nc.vector.tensor_add(dstT[:], t1[:], swp[:])

                    # V natural blocks for this quarter (4 transposes of 128x128)
                    for jb in range(nq * QW // 128, (nq + 1) * QW // 128):
                        bcols = slice(jb * 128, (jb + 1) * 128)
                        lcols = slice((jb % 4) * 128, (jb % 4 + 1) * 128)
                        tr = trpp.tile([128, 128], BF16, tag="tr", name="tr")
                        nc.tensor.transpose(tr[:], vT[:, bcols], eye_sb[:])
                        nc.scalar.copy(vNat[nq][:, lcols], tr[:])

            # ---------------- Phase 2+3: per query pair i (256 queries):
            # attention -> AllGather; o_proj runs one chunk behind so the
            # collective hides under attn(i+1) + o_proj(i-1) PE work.
            with (
                tc.tile_pool(name=f"e_sb{rep}", bufs=3) as ep,
                tc.tile_pool(name=f"att_sb{rep}", bufs=2) as asb,
                tc.tile_pool(name=f"ev_sb{rep}", bufs=3) as evp,
                tc.tile_pool(name=f"s_ps{rep}", bufs=3, space="PSUM") as spsp,
                tc.tile_pool(name=f"o_ps{rep}", bufs=2, space="PSUM") as opsp,
                tc.tile_pool(name=f"m_ps{rep}", bufs=1, space="PSUM") as mpsp,
                tc.tile_pool(name=f"op_ps{rep}", bufs=2, space="PSUM") as oppp,
            ):
                ag_ins = [
                    dramp.tile([DQ, 256], BF16, tag=f"agin{c}", name=f"agin{c}")
                    for c in range(NBLK // 2)
                ]
                ag_outs = [
                    dramp.tile(
                        [H, 256], BF16, tag=f"agout{c}", name=f"agout{c}",
                        addr_space="Shared",
                    )
                    for c in range(NBLK // 2)
                ]

                def attn_chunk(i: int):
                    q0 = i * 256
                    js = _pair_js(i)
                    L = len(js)
                    for h in range(HQ):
                        e_t = ep.tile(
                            [128, L * 256], BF16, tag="e", name="e_t"
                        )
                        oT = opsp.tile([128, 256], F32, tag="oT", name="oT")
                        sm = mpsp.tile([1, 256], F32, tag="sm", name="sm")

                        spans = []
                        for j in js:
                            left = (j == 0) or (j <= 2 * i <= j + 7)
                            right = (j == 0) or (j <= 2 * i + 1 <= j + 7)
                            qs = q0 if left else q0 + 128
                            qe = q0 + 256 if right else q0 + 128
                            spans.append((qs, qe))

                        def score(idx: int):
                            j = js[idx]
                            qs, qe = spans[idx]
                            w = qe - qs
                            ecols = slice(idx * 256, idx * 256 + w)
                            s_ps = spsp.tile([128, 256], F32, tag="sps", name="s_ps")
                            kq_, kc_ = j // 4, (j % 4) * 128
                            qq_ = qs // QW
                            nc.tensor.matmul(
                                s_ps[:, 0:w],
                                kTr[kq_][:, kc_ : kc_ + 128],
                                qTr[h][qq_][:, qs - qq_ * QW : qe - qq_ * QW],
                                start=True,
                                stop=True,
                            )
                            nc.scalar.activation(
                                e_t[:, ecols],
                                s_ps[:, 0:w],
                                mybir.ActivationFunctionType.Exp,
                                scale=SCALE,
                            )
                            if j == 2 * i:
                                nc.vector.tensor_mul(
                                    e_t[:, ecols], e_t[:, ecols], maskA_sb[:]
                                )
                            elif j == 2 * i + 1:
                                nc.vector.tensor_mul(
                                    e_t[:, ecols], e_t[:, ecols], tri_sb[:]
                                )

                        def av(idx: int):
                            j = js[idx]
                            qs, qe = spans[idx]
                            w = qe - qs
                            ecols = slice(idx * 256, idx * 256 + w)
                            st, sp_ = (idx == 0), (idx == L - 1)
                            nc.tensor.matmul(
                                oT[:, qs - q0 : qe - q0],
                                vNat[j // 4][:, (j % 4) * 128 : (j % 4 + 1) * 128],
                                e_t[:, ecols],
                                start=st,
                                stop=sp_,
                            )
                            nc.tensor.matmul(
                                sm[:, qs - q0 : qe - q0],
                                ones_sb[:],
                                e_t[:, ecols],
                                start=st,
                                stop=sp_,
                            )

                        # software pipeline: keep 2 score blocks in flight so
                        # the ACT exp round trip hides under PE work
                        score(0)
                        if L > 1:
                            score(1)
                        for idx in range(L):
                            if idx + 2 < L:
                                score(idx + 2)
                            av(idx)

                        r_sb = asb.tile([1, 256], F32, tag="r", name="r_sb")
                        nc.vector.reciprocal(r_sb[:], sm[:])
                        rb = asb.tile([128, 256], F32, tag="rb", name="rb")
                        nc.gpsimd.partition_broadcast(rb[:], r_sb[:])
                        at_c = asb.tile(
                            [128, 256], BF16, tag=f"at{h}", name=f"at{h}"
                        )
                        nc.vector.tensor_mul(at_c[:], oT[:], rb[:])
                        nc.sync.dma_start(
                            ag_ins[i][h * 128 : (h + 1) * 128, :], at_c[:]
                        )

                    if rs:
                        nc.gpsimd.collective_compute(
                            "AllGather",
                            mybir.AluOpType.bypass,
                            replica_groups=[list(range(N_CORES))],
                            ins=[ag_ins[i].opt()],
                            outs=[ag_outs[i].opt()],
                        )
                    else:
                        nc.sync.dma_start(ag_outs[i][0:DQ, :], ag_ins[i][:])

                def oproj_chunk(i: int):
                    q0 = i * 256
                    ps01 = [
                        oppp.tile([128, 512], F32, tag=f"op{sb}", name=f"op{sb}", bufs=1)
                        for sb in range(2)
                    ]
                    for c in range(KC):
                        ag_sb = evp.tile(
                            [128, 256], BF16, tag="ag_sb", name="ag_sb", bufs=6
                        )
                        nc.sync.dma_start(
                            ag_sb[:], ag_outs[i][c * 128 : (c + 1) * 128, :]
                        )
                        for sb in range(2):
                            nc.tensor.matmul(
                                ps01[sb][:],
                                ag_sb[:, sb * 128 : (sb + 1) * 128],
                                wo_t[c][:],
                                start=(c == 0),
                                stop=(c == KC - 1),
                            )
                    for sb in range(2):
                        ev = evp.tile([128, 512], F32, tag="ev", name="ev")
                        nc.vector.tensor_copy(ev[:], ps01[sb][:])
                        nc.sync.dma_start(
                            out[q0 + sb * 128 : q0 + (sb + 1) * 128, :], ev[:]
                        )

                for i in range(NBLK // 2):
                    attn_chunk(i)
                    if i >= 1:
                        oproj_chunk(i - 1)
                oproj_chunk(NBLK // 2 - 1)
    nc.compile()
    return nc


@functools.lru_cache(maxsize=1)
def _cached_nc():
    return build_nc(rs=True)


def _tables():
    pos = np.arange(S, dtype=np.float64)
    inv = 1.0 / (ROPE_BASE ** (np.arange(0, HD, 2, dtype=np.float64) / HD))  # [64]
    f = inv[:, None] * pos[None, :]                   # [64, S]
    cos = np.cos(f).astype(np.float32)
    sin = np.sin(f).astype(np.float32)
    cosF = np.concatenate([cos, cos], axis=0)         # [128, S]
    sinS = np.concatenate([-sin, sin], axis=0)        # [128, S]
    k_idx = np.arange(128)[:, None]
    q_idx = np.arange(128)[None, :]
    tri = (k_idx <= q_idx).astype(np.float32)         # [k, q] causal in-block
    eye = np.eye(128, dtype=np.float32)
    maskA = np.concatenate([tri, np.ones((128, 128), np.float32)], axis=1)
    return cosF, sinS, tri, eye, maskA


def _bf16(x: np.ndarray) -> np.ndarray:
    return np.ascontiguousarray(x).astype(ml_dtypes.bfloat16)


def kernel(hidden_states, wq, wk, wv, wo):
    nc = _cached_nc()
    hidT = _bf16(np.asarray(hidden_states, dtype=np.float32).reshape(S, H).T)
    cosF, sinS, tri, eye, maskA = _tables()
    in_maps = []
    for c in range(N_CORES):
        in_maps.append(
            {
                "hidT": hidT,
                "wq": _bf16(wq[:, c * DQ : (c + 1) * DQ]),
                "wk": _bf16(wk[:, c * HD : (c + 1) * HD]),
                "wv": _bf16(wv[:, c * HD : (c + 1) * HD]),
                "wo": _bf16(wo[:, c * DQ : (c + 1) * DQ]),
                "cosF": cosF,
                "sinS": sinS,
                "tri": _bf16(tri),
                "eye": _bf16(eye),
                "onescol": np.ones((128, 1), dtype=ml_dtypes.bfloat16),
                "maskA": _bf16(maskA),
            }
        )
    kw = dict(trace=True, **TRACE_KW) if TRACE else {}
    res = run_bass_kernel_spmd(nc, in_maps, core_ids=list(range(N_CORES)), **kw)
    global LAST_RESULTS
    LAST_RESULTS = res
    full = np.concatenate(
        [res.results[r]["out"] for r in range(N_CORES)], axis=1
    )
    return full.reshape(B, S, H)


# revision 9
# speedup vs baseline: 1.3244x; 1.1183x over previous
"""Trainium2 Bass kernel for nn_LlamaAttention_48816598286577.

Llama attention with block-streaming sparse mask (sink=1 block, local
window=8 blocks, BLOCK=128), B=1 S=2048 H=4096, 32 q heads / 8 kv heads,
head_dim 128, non-interleaved RoPE.

Sharding: tensor-parallel over heads across 8 cores (4 q heads + 1 kv
head per core). All matmul operands are bf16 (f32r measured ~2x slower
per row and ~4x slower LDWEIGHTS on HW); accumulation is f32 in PSUM.

Schedule: the S=2048 sequence is processed in 4 projection quarters.
After each quarter's QKV+RoPE, the two 256-query attention chunks it
unlocks run immediately, each followed by its bf16 AllGather; o_proj
for chunk i runs two chunks later, so every AllGather hides under
attention + o_proj + next-quarter PE work. DMA descriptor generation is
spread across the Sync/Scalar/Vector/GpSimd queues to avoid the
in-order SP queue serializing issue (565ns each).
"""

import functools
import numpy as np
import ml_dtypes

import concourse.bass as bass
import concourse.mybir as mybir
import concourse.tile as tile
from concourse import bacc
from concourse.bass_utils import run_bass_kernel_spmd

# problem constants (hardcoded per contract)
B, S, H = 1, 2048, 4096
NQ, NKV, HD = 32, 8, 128
BLOCK = 128
NBLK = S // BLOCK          # 16
SINK_BLOCKS = 1
LOCAL_BLOCKS = 8
ROPE_BASE = 10000.0
N_CORES = 8
HQ = NQ // N_CORES         # 4 q heads per core
DQ = HQ * HD               # 512 q columns per core
SCALE = 1.0 / float(np.sqrt(HD))

KC = H // 128              # 32 contraction chunks for projections
NQUART = 4                 # S split into 4 quarters of 512 for projections
QW = S // NQUART           # 512
NCH = NBLK // 2            # 8 attention chunks of 256 queries

F32 = mybir.dt.float32
BF16 = mybir.dt.bfloat16

# Opt-in profiling plumbing (off by default; harness never touches these).
TRACE = False
TRACE_KW: dict = {}
LAST_RESULTS = None


def _pair_js(i: int) -> list[int]:
    """Key blocks contributing to query pair i (blocks 2i, 2i+1)."""
    return sorted(set([0]) | set(range(max(0, 2 * i - 7), 2 * i + 2)))


def build_nc(rs: bool = True, repeat: int = 1):
    nc = bacc.Bacc(
        "TRN2", target_bir_lowering=False, debug=False, num_devices=N_CORES
    )
    hidT = nc.dram_tensor("hidT", [H, S], BF16, kind="ExternalInput").ap()
    wq = nc.dram_tensor("wq", [H, DQ], BF16, kind="ExternalInput").ap()
    wk = nc.dram_tensor("wk", [H, HD], BF16, kind="ExternalInput").ap()
    wv = nc.dram_tensor("wv", [H, HD], BF16, kind="ExternalInput").ap()
    wo = nc.dram_tensor("wo", [H, DQ], BF16, kind="ExternalInput").ap()
    cosF = nc.dram_tensor("cosF", [128, S], F32, kind="ExternalInput").ap()
    sinS = nc.dram_tensor("sinS", [128, S], F32, kind="ExternalInput").ap()
    tri = nc.dram_tensor("tri", [128, 128], BF16, kind="ExternalInput").ap()
    eye = nc.dram_tensor("eye", [128, 128], F32, kind="ExternalInput").ap()
    onescol = nc.dram_tensor("onescol", [128, 1], BF16, kind="ExternalInput").ap()
    maskA = nc.dram_tensor("maskA", [128, 256], BF16, kind="ExternalInput").ap()
    out = nc.dram_tensor("out", [S, DQ], F32, kind="ExternalOutput").ap()

    with tile.TileContext(nc) as tc:
      for rep in range(repeat):
        with (
            tc.tile_pool(name=f"persist{rep}", bufs=1) as pp,
            tc.tile_pool(name=f"dram{rep}", bufs=1, space="DRAM") as dramp,
            tc.tile_pool(name=f"sp{rep}", bufs=1) as sp,
            tc.tile_pool(name=f"stream{rep}", bufs=3) as stp,
            tc.tile_pool(name=f"e_sb{rep}", bufs=3) as ep,
            tc.tile_pool(name=f"att_sb{rep}", bufs=2) as asb,
            tc.tile_pool(name=f"ev_sb{rep}", bufs=3) as evp,
        ):
            qTr = [
                [
                    pp.tile([128, QW], BF16, tag=f"qTr{h}_{nq}", name=f"qTr{h}_{nq}")
                    for nq in range(NQUART)
                ]
                for h in range(HQ)
            ]
            kTr = [
                pp.tile([128, QW], BF16, tag=f"kTr{nq}", name=f"kTr{nq}")
                for nq in range(NQUART)
            ]
            vNat = [
                pp.tile([128, QW], BF16, tag=f"vNat{nq}", name=f"vNat{nq}")
                for nq in range(NQUART)
            ]
            tri_sb = pp.tile([128, 128], BF16, tag="tri", name="tri_sb")
            maskA_sb = pp.tile([128, 256], BF16, tag="maskA", name="maskA_sb")
            eye_sb = pp.tile([128, 128], F32, tag="eye", name="eye_sb")
            ones_sb = pp.tile([128, 1], BF16, tag="ones", name="ones_sb")
            nc.sync.dma_start(tri_sb[:], tri[:])
            nc.sync.dma_start(maskA_sb[:], maskA[:])
            nc.sync.dma_start(eye_sb[:], eye[:])
            nc.sync.dma_start(ones_sb[:], onescol[:])

            # CC-stream warmup collective, issued first on gpsimd so the
            # ~50us NRT stream init overlaps the first projection quarter.
            if rs:
                warm_in = dramp.tile([16, 16], BF16, tag="warm_in", name="warm_in")
                warm_out = dramp.tile(
                    [128, 16], BF16, tag="warm_out", name="warm_out",
                    addr_space="Shared",
                )
                warm_src = pp.tile([16, 16], BF16, tag="warm_src", name="warm_src")
                nc.vector.memset(warm_src[:], 0.0)
                nc.gpsimd.dma_start(warm_in[:], warm_src[:])
                nc.gpsimd.collective_compute(
                    "AllGather",
                    mybir.AluOpType.bypass,
                    replica_groups=[list(range(N_CORES))],
                    ins=[warm_in.opt()],
                    outs=[warm_out.opt()],
                )

            # weight prefetch: qkv weights issue on SP in consumption order
            # (interleaved per chunk, pacing the quarter-0 matmuls); wo on
            # gpsimd (needed only from the second slot on).
            wo_t = []
            for c in range(KC):
                tw = sp.tile([128, DQ], BF16, tag=f"woc{c}", name=f"woc{c}")
                nc.gpsimd.dma_start(tw[:], wo[c * 128 : (c + 1) * 128, :])
                wo_t.append(tw)
            wq_t, wk_t, wv_t = [], [], []
            for c in range(KC):
                crow = slice(c * 128, (c + 1) * 128)
                tq = sp.tile([128, DQ], BF16, tag=f"wqc{c}", name=f"wqc{c}")
                tk = sp.tile([128, HD], BF16, tag=f"wkc{c}", name=f"wkc{c}")
                tv = sp.tile([128, HD], BF16, tag=f"wvc{c}", name=f"wvc{c}")
                nc.sync.dma_start(tq[:], wq[crow, :])
                nc.sync.dma_start(tk[:], wk[crow, :])
                nc.sync.dma_start(tv[:], wv[crow, :])
                wq_t.append(tq)
                wk_t.append(tk)
                wv_t.append(tv)

            vT = sp.tile([128, S], F32, tag="vT", name="vT")
            ag_ins = [
                dramp.tile([DQ, 256], BF16, tag=f"agin{c}", name=f"agin{c}")
                for c in range(NCH)
            ]
            ag_outs = [
                dramp.tile(
                    [H, 256], BF16, tag=f"agout{c}", name=f"agout{c}",
                    addr_space="Shared",
                )
                for c in range(NCH)
            ]

            def attn_chunk(i: int, apool):
                q0 = i * 256
                js = _pair_js(i)
                L = len(js)
                for h in range(HQ):
                    e_t = ep.tile([128, L * 256], BF16, tag="e", name="e_t")
                    oT = apool.tile([128, 256], F32, tag="oT", name="oT", bufs=2)
                    sm = apool.tile([1, 256], F32, tag="sm", name="sm", bufs=1)

                    spans = []
                    for j in js:
                        left = (j == 0) or (j <= 2 * i <= j + 7)
                        right = (j == 0) or (j <= 2 * i + 1 <= j + 7)
                        qs = q0 if left else q0 + 128
                        qe = q0 + 256 if right else q0 + 128
                        spans.append((qs, qe))

                    def score(idx: int):
                        j = js[idx]
                        qs, qe = spans[idx]
                        w = qe - qs
                        ecols = slice(idx * 256, idx * 256 + w)
                        s_ps = apool.tile(
                            [128, 256], F32, tag="sps", name="s_ps", bufs=3
                        )
                        kq_, kc_ = j // 4, (j % 4) * 128
                        qq_ = qs // QW
                        nc.tensor.matmul(
                            s_ps[:, 0:w],
                            kTr[kq_][:, kc_ : kc_ + 128],
                            qTr[h][qq_][:, qs - qq_ * QW : qe - qq_ * QW],
                            start=True,
                            stop=True,
                        )
                        nc.scalar.activation(
                            e_t[:, ecols],
                            s_ps[:, 0:w],
                            mybir.ActivationFunctionType.Exp,
                            scale=SCALE,
                        )
                        if j == 2 * i:
                            nc.vector.tensor_mul(
                                e_t[:, ecols], e_t[:, ecols], maskA_sb[:]
                            )
                        elif j == 2 * i + 1:
                            nc.vector.tensor_mul(
                                e_t[:, ecols], e_t[:, ecols], tri_sb[:]
                            )

                    def av(idx: int):
                        j = js[idx]
                        qs, qe = spans[idx]
                        w = qe - qs
                        ecols = slice(idx * 256, idx * 256 + w)
                        st, sp_ = (idx == 0), (idx == L - 1)
                        nc.tensor.matmul(
                            oT[:, qs - q0 : qe - q0],
                            vNat[j // 4][:, (j % 4) * 128 : (j % 4 + 1) * 128],
                            e_t[:, ecols],
                            start=st,
                            stop=sp_,
                        )
                        nc.tensor.matmul(
                            sm[:, qs - q0 : qe - q0],
                            ones_sb[:],
                            e_t[:, ecols],
                            start=st,
                            stop=sp_,
                        )

                    score(0)
                    if L > 1:
                        score(1)
                    for idx in range(L):
                        if idx + 2 < L:
                            score(idx + 2)
                        av(idx)

                    r_sb = asb.tile([1, 256], F32, tag="r", name="r_sb")
                    nc.vector.reciprocal(r_sb[:], sm[:])
                    rb = asb.tile([128, 256], F32, tag="rb", name="rb")
                    nc.gpsimd.partition_broadcast(rb[:], r_sb[:])
                    at_c = asb.tile([128, 256], BF16, tag=f"at{h}", name=f"at{h}")
                    nc.vector.tensor_mul(at_c[:], oT[:], rb[:])
                    nc.sync.dma_start(
                        ag_ins[i][h * 128 : (h + 1) * 128, :], at_c[:]
                    )

                if rs:
                    nc.gpsimd.collective_compute(
                        "AllGather",
                        mybir.AluOpType.bypass,
                        replica_groups=[list(range(N_CORES))],
                        ins=[ag_ins[i].opt()],
                        outs=[ag_outs[i].opt()],
                    )
                else:
                    nc.sync.dma_start(ag_outs[i][0:DQ, :], ag_ins[i][:])

            def oproj_chunk(i: int, oppool):
                q0 = i * 256
                ps01 = [
                    oppool.tile(
                        [128, 512], F32, tag=f"op{sb}", name=f"op{sb}", bufs=1
                    )
                    for sb in range(2)
                ]
                for c in range(KC):
                    ag_sb = evp.tile(
                        [128, 256], BF16, tag="ag_sb", name="ag_sb", bufs=6
                    )
                    nc.sync.dma_start(
                        ag_sb[:], ag_outs[i][c * 128 : (c + 1) * 128, :]
                    )
                    for sb in range(2):
                        nc.tensor.matmul(
                            ps01[sb][:],
                            ag_sb[:, sb * 128 : (sb + 1) * 128],
                            wo_t[c][:],
                            start=(c == 0),
                            stop=(c == KC - 1),
                        )
                for sb in range(2):
                    ev = evp.tile([128, 512], F32, tag="ev", name="ev")
                    nc.vector.tensor_copy(ev[:], ps01[sb][:])
                    nc.sync.dma_start(
                        out[q0 + sb * 128 : q0 + (sb + 1) * 128, :], ev[:]
                    )

            for nq in range(NQUART):
                ncols = slice(nq * QW, (nq + 1) * QW)
                with tc.tile_pool(
                    name=f"qps{rep}_{nq}", bufs=1, space="PSUM"
                ) as qpool:
                    ps_q = [
                        qpool.tile([128, QW], F32, tag=f"psq{h}", name=f"psq{h}")
                        for h in range(HQ)
                    ]
                    ps_k = qpool.tile([128, QW], F32, tag="psk", name="ps_k")
                    ps_v = qpool.tile([128, QW], F32, tag="psv", name="ps_v")
                    # quarter 0's hid stream issues on ACT so it doesn't sit
                    # behind the 96 weight DMAs on the SP queue
                    hid_eng = nc.scalar if nq == 0 else nc.sync
                    for c in range(KC):
                        crow = slice(c * 128, (c + 1) * 128)
                        hid_c = stp.tile([128, QW], BF16, tag="hid", name="hid_c")
                        hid_eng.dma_start(hid_c[:], hidT[crow, ncols])
                        st, sp_ = (c == 0), (c == KC - 1)
                        for h in range(HQ):
                            nc.tensor.matmul(
                                ps_q[h][:],
                                wq_t[c][:, h * HD : (h + 1) * HD],
                                hid_c[:],
                                start=st,
                                stop=sp_,
                            )
                        nc.tensor.matmul(
                            ps_k[:], wk_t[c][:], hid_c[:], start=st, stop=sp_
                        )
                        nc.tensor.matmul(
                            ps_v[:], wv_t[c][:], hid_c[:], start=st, stop=sp_
                        )

                    cos_sb = stp.tile([128, QW], F32, tag="cos", name="cos_sb", bufs=2)
                    sin_sb = stp.tile([128, QW], F32, tag="sin", name="sin_sb", bufs=2)
                    nc.sync.dma_start(cos_sb[:], cosF[:, ncols])
                    nc.sync.dma_start(sin_sb[:], sinS[:, ncols])

                    # Evacuate PSUM on ACT (q0 first so the next quarter's
                    # first matmul unblocks earliest), then RoPE on DVE with
                    # the half-swap via gpsimd-issued SBUF DMAs.
                    rope_list = [(ps_q[h], qTr[h][nq], f"q{h}") for h in range(HQ)]
                    rope_list.append((ps_k, kTr[nq], "k"))
                    raws = []
                    for ps_x, dstT, tag in rope_list:
                        raw = sp.tile([128, QW], F32, tag=f"raw{tag}", name=f"raw{tag}")
                        nc.scalar.copy(raw[:], ps_x[:])
                        raws.append(raw)
                    nc.scalar.copy(vT[:, ncols], ps_v[:])
                    swps = []
                    for raw, (_, _, tag) in zip(raws, rope_list):
                        swp = sp.tile([128, QW], F32, tag=f"swp{tag}", name=f"swp{tag}")
                        nc.gpsimd.dma_start(swp[0:64, :], raw[64:128, :])
                        nc.gpsimd.dma_start(swp[64:128, :], raw[0:64, :])
                        swps.append(swp)
                    t1s = []
                    for raw, (_, _, tag) in zip(raws, rope_list):
                        t1 = sp.tile([128, QW], F32, tag=f"t1{tag}", name=f"t1{tag}")
                        nc.vector.tensor_mul(t1[:], raw[:], cos_sb[:])
                        t1s.append(t1)
                    for t1, swp, (_, dstT, _) in zip(t1s, swps, rope_list):
                        nc.vector.tensor_mul(swp[:], swp[:], sin_sb[:])
                        nc.vector.tensor_add(dstT[:], t1[:], swp[:])

                    # V natural blocks for this quarter (4 transposes)
                    with tc.tile_pool(
                        name=f"trp{rep}_{nq}", bufs=2, space="PSUM"
                    ) as trpool:
                        for jb in range(nq * QW // 128, (nq + 1) * QW // 128):
                            bcols = slice(jb * 128, (jb + 1) * 128)
                            lcols = slice((jb % 4) * 128, (jb % 4 + 1) * 128)
                            tr = trpool.tile([128, 128], F32, tag="tr", name="tr")
                            nc.tensor.transpose(tr[:], vT[:, bcols], eye_sb[:])
                            nc.scalar.copy(vNat[nq][:, lcols], tr[:])

                # attention chunks unlocked by this quarter; o_proj lags by
                # 2 chunks so each AllGather hides under subsequent PE work
                with tc.tile_pool(
                    name=f"aps{rep}_{nq}", bufs=1, space="PSUM"
                ) as apool:
                    attn_chunk(2 * nq, apool)
                    if 2 * nq - 2 >= 0:
                        with tc.tile_pool(
                            name=f"ops{rep}_{nq}a", bufs=1, space="PSUM"
                        ) as oppool:
                            oproj_chunk(2 * nq - 2, oppool)
                    attn_chunk(2 * nq + 1, apool)
                    if 2 * nq - 1 >= 0:
                        with tc.tile_pool(
                            name=f"ops{rep}_{nq}b", bufs=1, space="PSUM"
                        ) as oppool:
                            oproj_chunk(2 * nq - 1, oppool)

            with tc.tile_pool(
                name=f"ops{rep}_t6", bufs=1, space="PSUM"
            ) as oppool:
                oproj_chunk(NCH - 2, oppool)
            with tc.tile_pool(
                name=f"ops{rep}_t7", bufs=1, space="PSUM"
            ) as oppool:
                oproj_chunk(NCH - 1, oppool)
    nc.compile()
    return nc


@functools.lru_cache(maxsize=1)
def _cached_nc():
    return build_nc(rs=True)


def _tables():
    pos = np.arange(S, dtype=np.float64)
    inv = 1.0 / (ROPE_BASE ** (np.arange(0, HD, 2, dtype=np.float64) / HD))  # [64]
    f = inv[:, None] * pos[None, :]                   # [64, S]
    cos = np.cos(f).astype(np.float32)
    sin = np.sin(f).astype(np.float32)
    cosF = np.concatenate([cos, cos], axis=0)         # [128, S]
    sinS = np.concatenate([-sin, sin], axis=0)        # [128, S]
    k_idx = np.arange(128)[:, None]
    q_idx = np.arange(128)[None, :]
    tri = (k_idx <= q_idx).astype(np.float32)         # [k, q] causal in-block
    eye = np.eye(128, dtype=np.float32)
    maskA = np.concatenate([tri, np.ones((128, 128), np.float32)], axis=1)
    return cosF, sinS, tri, eye, maskA


def _bf16(x: np.ndarray) -> np.ndarray:
    return np.ascontiguousarray(x).astype(ml_dtypes.bfloat16)


def kernel(hidden_states, wq, wk, wv, wo):
    nc = _cached_nc()
    hidT = _bf16(np.asarray(hidden_states, dtype=np.float32).reshape(S, H).T)
    cosF, sinS, tri, eye, maskA = _tables()
    in_maps = []
    for c in range(N_CORES):
        in_maps.append(
            {
                "hidT": hidT,
                "wq": _bf16(wq[:, c * DQ : (c + 1) * DQ]),
                "wk": _bf16(wk[:, c * HD : (c + 1) * HD]),
                "wv": _bf16(wv[:, c * HD : (c + 1) * HD]),
                "wo": _bf16(wo[:, c * DQ : (c + 1) * DQ]),
                "cosF": cosF,
                "sinS": sinS,
                "tri": _bf16(tri),
                "eye": eye,
                "onescol": np.ones((128, 1), dtype=ml_dtypes.bfloat16),
                "maskA": _bf16(maskA),
            }
        )
    kw = dict(trace=True, **TRACE_KW) if TRACE else {}
    res = run_bass_kernel_spmd(nc, in_maps, core_ids=list(range(N_CORES)), **kw)
    global LAST_RESULTS
    LAST_RESULTS = res
    full = np.concatenate(
        [res.results[r]["out"] for r in range(N_CORES)], axis=1
    )
    return full.reshape(B, S, H)


# revision 13
# speedup vs baseline: 1.3931x; 1.0519x over previous
"""Trainium2 Bass kernel for nn_LlamaAttention_48816598286577.

Llama attention with block-streaming sparse mask (sink=1 block, local
window=8 blocks, BLOCK=128), B=1 S=2048 H=4096, 32 q heads / 8 kv heads,
head_dim 128, non-interleaved RoPE.

Sharding: tensor-parallel over heads across 8 cores (4 q heads + 1 kv
head per core). All matmul operands are bf16 (f32r measured ~2x slower
per row and ~4x slower LDWEIGHTS on HW); accumulation is f32 in PSUM.

Schedule: the S=2048 sequence is processed in 4 projection quarters.
After each quarter's QKV+RoPE, the two 256-query attention chunks it
unlocks run immediately, each followed by its bf16 AllGather; o_proj
for chunk i runs two chunks later, so every AllGather hides under
attention + o_proj + next-quarter PE work. DMA descriptor generation is
spread across the Sync/Scalar/Vector/GpSimd queues to avoid the
in-order SP queue serializing issue (565ns each).
"""

import functools
import numpy as np
import ml_dtypes

import concourse.bass as bass
import concourse.mybir as mybir
import concourse.tile as tile
from concourse import bacc
from concourse.bass_utils import run_bass_kernel_spmd

# problem constants (hardcoded per contract)
B, S, H = 1, 2048, 4096
NQ, NKV, HD = 32, 8, 128
BLOCK = 128
NBLK = S // BLOCK          # 16
SINK_BLOCKS = 1
LOCAL_BLOCKS = 8
ROPE_BASE = 10000.0
N_CORES = 8
HQ = NQ // N_CORES         # 4 q heads per core
DQ = HQ * HD               # 512 q columns per core
SCALE = 1.0 / float(np.sqrt(HD))

KC = H // 128              # 32 contraction chunks for projections
NQUART = 4                 # S split into 4 quarters of 512 for projections
QW = S // NQUART           # 512
NCH = NBLK // 2            # 8 attention chunks of 256 queries

F32 = mybir.dt.float32
BF16 = mybir.dt.bfloat16

# Opt-in profiling plumbing (off by default; harness never touches these).
TRACE = False
TRACE_KW: dict = {}
LAST_RESULTS = None


def _pair_js(i: int) -> list[int]:
    """Key blocks contributing to query pair i (blocks 2i, 2i+1)."""
    return sorted(set([0]) | set(range(max(0, 2 * i - 7), 2 * i + 2)))


def build_nc(rs: bool = True, repeat: int = 1):
    nc = bacc.Bacc(
        "TRN2", target_bir_lowering=False, debug=False, num_devices=N_CORES
    )
    hidT = nc.dram_tensor("hidT", [H, S], BF16, kind="ExternalInput").ap()
    wq = nc.dram_tensor("wq", [H, DQ], BF16, kind="ExternalInput").ap()
    wk = nc.dram_tensor("wk", [H, HD], BF16, kind="ExternalInput").ap()
    wv = nc.dram_tensor("wv", [H, HD], BF16, kind="ExternalInput").ap()
    wo = nc.dram_tensor("wo", [H, DQ], BF16, kind="ExternalInput").ap()
    cosF = nc.dram_tensor("cosF", [128, S], F32, kind="ExternalInput").ap()
    sinS = nc.dram_tensor("sinS", [128, S], F32, kind="ExternalInput").ap()
    tri = nc.dram_tensor("tri", [128, 128], BF16, kind="ExternalInput").ap()
    eye = nc.dram_tensor("eye", [128, 128], F32, kind="ExternalInput").ap()
    onescol = nc.dram_tensor("onescol", [128, 1], BF16, kind="ExternalInput").ap()
    maskA = nc.dram_tensor("maskA", [128, 256], BF16, kind="ExternalInput").ap()
    out = nc.dram_tensor("out", [S, DQ], F32, kind="ExternalOutput").ap()

    with tile.TileContext(nc) as tc:
      for rep in range(repeat):
        with (
            tc.tile_pool(name=f"persist{rep}", bufs=1) as pp,
            tc.tile_pool(name=f"dram{rep}", bufs=1, space="DRAM") as dramp,
            tc.tile_pool(name=f"sp{rep}", bufs=1) as sp,
            tc.tile_pool(name=f"stream{rep}", bufs=3) as stp,
            tc.tile_pool(name=f"e_sb{rep}", bufs=3) as ep,
            tc.tile_pool(name=f"att_sb{rep}", bufs=2) as asb,
            tc.tile_pool(name=f"ev_sb{rep}", bufs=3) as evp,
        ):
            qTr = [
                [
                    pp.tile([128, QW], BF16, tag=f"qTr{h}_{nq}", name=f"qTr{h}_{nq}")
                    for nq in range(NQUART)
                ]
                for h in range(HQ)
            ]
            kTr = [
                pp.tile([128, QW], BF16, tag=f"kTr{nq}", name=f"kTr{nq}")
                for nq in range(NQUART)
            ]
            vNat = [
                pp.tile([128, QW], BF16, tag=f"vNat{nq}", name=f"vNat{nq}")
                for nq in range(NQUART)
            ]
            tri_sb = pp.tile([128, 128], BF16, tag="tri", name="tri_sb")
            maskA_sb = pp.tile([128, 256], BF16, tag="maskA", name="maskA_sb")
            eye_sb = pp.tile([128, 128], F32, tag="eye", name="eye_sb")
            ones_sb = pp.tile([128, 1], BF16, tag="ones", name="ones_sb")
            nc.sync.dma_start(tri_sb[:], tri[:])
            nc.sync.dma_start(maskA_sb[:], maskA[:])
            nc.sync.dma_start(eye_sb[:], eye[:])
            nc.sync.dma_start(ones_sb[:], onescol[:])

            # CC-stream warmup collective, issued first on gpsimd so the
            # ~50us NRT stream init overlaps the first projection quarter.
            if rs:
                warm_in = dramp.tile([16, 16], BF16, tag="warm_in", name="warm_in")
                warm_out = dramp.tile(
                    [128, 16], BF16, tag="warm_out", name="warm_out",
                    addr_space="Shared",
                )
                warm_src = pp.tile([16, 16], BF16, tag="warm_src", name="warm_src")
                nc.vector.memset(warm_src[:], 0.0)
                nc.gpsimd.dma_start(warm_in[:], warm_src[:])
                nc.gpsimd.collective_compute(
                    "AllGather",
                    mybir.AluOpType.bypass,
                    replica_groups=[list(range(N_CORES))],
                    ins=[warm_in.opt()],
                    outs=[warm_out.opt()],
                )

            # weight prefetch: qkv weights issue on SP in consumption order
            # (interleaved per chunk, pacing the quarter-0 matmuls); wo on
            # gpsimd (needed only from the second slot on).
            wo_t = []
            for c in range(KC):
                tw = sp.tile([128, DQ], BF16, tag=f"woc{c}", name=f"woc{c}")
                nc.gpsimd.dma_start(tw[:], wo[c * 128 : (c + 1) * 128, :])
                wo_t.append(tw)
            wq_t, wk_t, wv_t = [], [], []
            for c in range(KC):
                crow = slice(c * 128, (c + 1) * 128)
                tq = sp.tile([128, DQ], BF16, tag=f"wqc{c}", name=f"wqc{c}")
                tk = sp.tile([128, HD], BF16, tag=f"wkc{c}", name=f"wkc{c}")
                tv = sp.tile([128, HD], BF16, tag=f"wvc{c}", name=f"wvc{c}")
                nc.sync.dma_start(tq[:], wq[crow, :])
                nc.sync.dma_start(tk[:], wk[crow, :])
                nc.sync.dma_start(tv[:], wv[crow, :])
                wq_t.append(tq)
                wk_t.append(tk)
                wv_t.append(tv)

            vT = sp.tile([128, S], F32, tag="vT", name="vT")
            ag_ins = [
                dramp.tile([DQ, 256], BF16, tag=f"agin{c}", name=f"agin{c}")
                for c in range(NCH)
            ]
            ag_outs = [
                dramp.tile(
                    [H, 256], BF16, tag=f"agout{c}", name=f"agout{c}",
                    addr_space="Shared",
                )
                for c in range(NCH)
            ]

            def attn_chunk(i: int, apool):
                q0 = i * 256
                js = _pair_js(i)
                L = len(js)
                for h in range(HQ):
                    e_t = ep.tile([128, L * 256], BF16, tag="e", name="e_t")
                    oT = apool.tile([128, 256], F32, tag="oT", name="oT", bufs=2)
                    sm = apool.tile([1, 256], F32, tag="sm", name="sm", bufs=1)

                    spans = []
                    for j in js:
                        left = (j == 0) or (j <= 2 * i <= j + 7)
                        right = (j == 0) or (j <= 2 * i + 1 <= j + 7)
                        qs = q0 if left else q0 + 128
                        qe = q0 + 256 if right else q0 + 128
                        spans.append((qs, qe))

                    def score(idx: int):
                        j = js[idx]
                        qs, qe = spans[idx]
                        w = qe - qs
                        ecols = slice(idx * 256, idx * 256 + w)
                        s_ps = apool.tile(
                            [128, 256], F32, tag="sps", name="s_ps", bufs=3
                        )
                        kq_, kc_ = j // 4, (j % 4) * 128
                        qq_ = qs // QW
                        nc.tensor.matmul(
                            s_ps[:, 0:w],
                            kTr[kq_][:, kc_ : kc_ + 128],
                            qTr[h][qq_][:, qs - qq_ * QW : qe - qq_ * QW],
                            start=True,
                            stop=True,
                        )
                        nc.scalar.activation(
                            e_t[:, ecols],
                            s_ps[:, 0:w],
                            mybir.ActivationFunctionType.Exp,
                            scale=SCALE,
                        )
                        if j == 2 * i:
                            nc.vector.tensor_mul(
                                e_t[:, ecols], e_t[:, ecols], maskA_sb[:]
                            )
                        elif j == 2 * i + 1:
                            nc.vector.tensor_mul(
                                e_t[:, ecols], e_t[:, ecols], tri_sb[:]
                            )

                    def av(idx: int):
                        j = js[idx]
                        qs, qe = spans[idx]
                        w = qe - qs
                        ecols = slice(idx * 256, idx * 256 + w)
                        st, sp_ = (idx == 0), (idx == L - 1)
                        nc.tensor.matmul(
                            oT[:, qs - q0 : qe - q0],
                            vNat[j // 4][:, (j % 4) * 128 : (j % 4 + 1) * 128],
                            e_t[:, ecols],
                            start=st,
                            stop=sp_,
                        )
                        nc.tensor.matmul(
                            sm[:, qs - q0 : qe - q0],
                            ones_sb[:],
                            e_t[:, ecols],
                            start=st,
                            stop=sp_,
                        )

                    score(0)
                    if L > 1:
                        score(1)
                    for idx in range(L):
                        if idx + 2 < L:
                            score(idx + 2)
                        av(idx)

                    r_sb = asb.tile([1, 256], F32, tag="r", name="r_sb")
                    nc.vector.reciprocal_approx_fast(r_sb[:], sm[:])
                    rb = asb.tile([128, 256], F32, tag="rb", name="rb")
                    nc.gpsimd.partition_broadcast(rb[:], r_sb[:])
                    at_c = asb.tile([128, 256], BF16, tag=f"at{h}", name=f"at{h}")
                    nc.vector.tensor_mul(at_c[:], oT[:], rb[:])
                    nc.sync.dma_start(
                        ag_ins[i][h * 128 : (h + 1) * 128, :], at_c[:]
                    )

                if rs:
                    nc.gpsimd.collective_compute(
                        "AllGather",
                        mybir.AluOpType.bypass,
                        replica_groups=[list(range(N_CORES))],
                        ins=[ag_ins[i].opt()],
                        outs=[ag_outs[i].opt()],
                    )
                else:
                    nc.sync.dma_start(ag_outs[i][0:DQ, :], ag_ins[i][:])

            def oproj_chunk(i: int, oppool):
                q0 = i * 256
                ps01 = [
                    oppool.tile(
                        [128, 512], F32, tag=f"op{sb}", name=f"op{sb}", bufs=1
                    )
                    for sb in range(2)
                ]
                # ag loads + out writes issue on gpsimd: they depend on the
                # collective, and on the SP queue a hoisted one head-of-line
                # blocks the next quarter's hid stream behind the AllGather
                for c in range(KC):
                    ag_sb = evp.tile(
                        [128, 256], BF16, tag="ag_sb", name="ag_sb", bufs=6
                    )
                    nc.gpsimd.dma_start(
                        ag_sb[:], ag_outs[i][c * 128 : (c + 1) * 128, :]
                    )
                    for sb in range(2):
                        nc.tensor.matmul(
                            ps01[sb][:],
                            ag_sb[:, sb * 128 : (sb + 1) * 128],
                            wo_t[c][:],
                            start=(c == 0),
                            stop=(c == KC - 1),
                        )
                for sb in range(2):
                    ev = evp.tile([128, 512], F32, tag="ev", name="ev")
                    nc.scalar.copy(ev[:], ps01[sb][:])
                    nc.gpsimd.dma_start(
                        out[q0 + sb * 128 : q0 + (sb + 1) * 128, :], ev[:]
                    )

            for nq in range(NQUART):
                ncols = slice(nq * QW, (nq + 1) * QW)
                with tc.tile_pool(
                    name=f"qps{rep}_{nq}", bufs=1, space="PSUM"
                ) as qpool:
                    ps_q = [
                        qpool.tile([128, QW], F32, tag=f"psq{h}", name=f"psq{h}")
                        for h in range(HQ)
                    ]
                    ps_k = qpool.tile([128, QW], F32, tag="psk", name="ps_k")
                    ps_v = qpool.tile([128, QW], F32, tag="psv", name="ps_v")
                    # quarter 0's hid stream issues on ACT so it doesn't sit
                    # behind the 96 weight DMAs on the SP queue
                    hid_eng = nc.scalar if nq == 0 else nc.sync
                    for c in range(KC):
                        crow = slice(c * 128, (c + 1) * 128)
                        hid_c = stp.tile([128, QW], BF16, tag="hid", name="hid_c")
                        hid_eng.dma_start(hid_c[:], hidT[crow, ncols])
                        st, sp_ = (c == 0), (c == KC - 1)
                        for h in range(HQ):
                            nc.tensor.matmul(
                                ps_q[h][:],
                                wq_t[c][:, h * HD : (h + 1) * HD],
                                hid_c[:],
                                start=st,
                                stop=sp_,
                            )
                        nc.tensor.matmul(
                            ps_k[:], wk_t[c][:], hid_c[:], start=st, stop=sp_
                        )
                        nc.tensor.matmul(
                            ps_v[:], wv_t[c][:], hid_c[:], start=st, stop=sp_
                        )

                    cos_sb = stp.tile([128, QW], F32, tag="cos", name="cos_sb", bufs=2)
                    sin_sb = stp.tile([128, QW], F32, tag="sin", name="sin_sb", bufs=2)
                    nc.sync.dma_start(cos_sb[:], cosF[:, ncols])
                    nc.sync.dma_start(sin_sb[:], sinS[:, ncols])

                    # Evacuate PSUM on ACT and RoPE on DVE, grouped per
                    # tensor with q0 first then k: the first attention chunk
                    # needs qTr[0] and the fresh kTr earliest. swp DMAs issue
                    # on ACT right after the raw copy they read.
                    rope_list = [(ps_q[0], qTr[0][nq], "q0"), (ps_k, kTr[nq], "k")]
                    rope_list += [
                        (ps_q[h], qTr[h][nq], f"q{h}") for h in range(1, HQ)
                    ]
                    for ps_x, dstT, tag in rope_list:
                        raw = sp.tile([128, QW], F32, tag=f"raw{tag}", name=f"raw{tag}")
                        nc.scalar.copy(raw[:], ps_x[:])
                        swp = sp.tile([128, QW], F32, tag=f"swp{tag}", name=f"swp{tag}")
                        nc.scalar.dma_start(swp[0:64, :], raw[64:128, :])
                        nc.scalar.dma_start(swp[64:128, :], raw[0:64, :])
                        t1 = sp.tile([128, QW], F32, tag=f"t1{tag}", name=f"t1{tag}")
                        nc.vector.tensor_mul(t1[:], raw[:], cos_sb[:])
                        nc.vector.tensor_mul(swp[:], swp[:], sin_sb[:])
                        nc.vector.tensor_add(dstT[:], t1[:], swp[:])
                    nc.scalar.copy(vT[:, ncols], ps_v[:])

                    # V natural blocks for this quarter (4 transposes)
                    with tc.tile_pool(
                        name=f"trp{rep}_{nq}", bufs=2, space="PSUM"
                    ) as trpool:
                        for jb in range(nq * QW // 128, (nq + 1) * QW // 128):
                            bcols = slice(jb * 128, (jb + 1) * 128)
                            lcols = slice((jb % 4) * 128, (jb % 4 + 1) * 128)
                            tr = trpool.tile([128, 128], F32, tag="tr", name="tr")
                            nc.tensor.transpose(tr[:], vT[:, bcols], eye_sb[:])
                            nc.scalar.copy(vNat[nq][:, lcols], tr[:])

                # attention chunks unlocked by this quarter; o_proj lags by
                # 2 chunks so each AllGather hides under subsequent PE work.
                # The last slot runs chunk 7 before 6 so AG7 is covered by
                # attn(6) + o_proj work instead of sticking out as a tail.
                ca, cb = (2 * nq, 2 * nq + 1) if nq < 3 else (7, 6)
                with tc.tile_pool(
                    name=f"aps{rep}_{nq}", bufs=1, space="PSUM"
                ) as apool:
                    attn_chunk(ca, apool)
                    if 2 * nq - 2 >= 0:
                        with tc.tile_pool(
                            name=f"ops{rep}_{nq}a", bufs=1, space="PSUM"
                        ) as oppool:
                            oproj_chunk(2 * nq - 2, oppool)
                    attn_chunk(cb, apool)
                    if 2 * nq - 1 >= 0:
                        with tc.tile_pool(
                            name=f"ops{rep}_{nq}b", bufs=1, space="PSUM"
                        ) as oppool:
                            oproj_chunk(2 * nq - 1, oppool)

            with tc.tile_pool(
                name=f"ops{rep}_t6", bufs=1, space="PSUM"
            ) as oppool:
                oproj_chunk(NCH - 2, oppool)
            with tc.tile_pool(
                name=f"ops{rep}_t7", bufs=1, space="PSUM"
            ) as oppool:
                oproj_chunk(NCH - 1, oppool)
    nc.compile()
    return nc


@functools.lru_cache(maxsize=1)
def _cached_nc():
    return build_nc(rs=True)


def _tables():
    pos = np.arange(S, dtype=np.float64)
    inv = 1.0 / (ROPE_BASE ** (np.arange(0, HD, 2, dtype=np.float64) / HD))  # [64]
    f = inv[:, None] * pos[None, :]                   # [64, S]
    cos = np.cos(f).astype(np.float32)
    sin = np.sin(f).astype(np.float32)
    cosF = np.concatenate([cos, cos], axis=0)         # [128, S]
    sinS = np.concatenate([-sin, sin], axis=0)        # [128, S]
    k_idx = np.arange(128)[:, None]
    q_idx = np.arange(128)[None, :]
    tri = (k_idx <= q_idx).astype(np.float32)         # [k, q] causal in-block
    eye = np.eye(128, dtype=np.float32)
    maskA = np.concatenate([tri, np.ones((128, 128), np.float32)], axis=1)
    return cosF, sinS, tri, eye, maskA


def _bf16(x: np.ndarray) -> np.ndarray:
    return np.ascontiguousarray(x).astype(ml_dtypes.bfloat16)


def kernel(hidden_states, wq, wk, wv, wo):
    nc = _cached_nc()
    hidT = _bf16(np.asarray(hidden_states, dtype=np.float32).reshape(S, H).T)
    cosF, sinS, tri, eye, maskA = _tables()
    in_maps = []
    for c in range(N_CORES):
        in_maps.append(
            {
                "hidT": hidT,
                "wq": _bf16(wq[:, c * DQ : (c + 1) * DQ]),
                "wk": _bf16(wk[:, c * HD : (c + 1) * HD]),
                "wv": _bf16(wv[:, c * HD : (c + 1) * HD]),
                "wo": _bf16(wo[:, c * DQ : (c + 1) * DQ]),
                "cosF": cosF,
                "sinS": sinS,
                "tri": _bf16(tri),
                "eye": eye,
                "onescol": np.ones((128, 1), dtype=ml_dtypes.bfloat16),
                "maskA": _bf16(maskA),
            }
        )
    kw = dict(trace=True, **TRACE_KW) if TRACE else {}
    res = run_bass_kernel_spmd(nc, in_maps, core_ids=list(range(N_CORES)), **kw)
    global LAST_RESULTS
    LAST_RESULTS = res
    full = np.concatenate(
        [res.results[r]["out"] for r in range(N_CORES)], axis=1
    )
    return full.reshape(B, S, H)
